# revision 21
# baseline (speedup 1.0000x reference)
"""Trainium2 Bass kernel for CRF score expansion.

Computes crf_scores[b, l, i, j] = emission[b, l, j] + transition[i, j]
for emission [32, 512, 64] f32 and transition [64, 64] f32, output
[32, 512, 64, 64] f32 (256 MB).

Sharding: data-parallel over the batch axis — 8 NeuronCores, 4 batches
(2048 (b,l) rows) per core; transition is replicated. No collectives.

Per-core kernel. Row→partition mapping gives partition p the 16
consecutive rows [16p, 16p+16), so:
  - the whole 512 KB emission shard loads in ONE DMA with one
    contiguous 4 KB descriptor per partition (the original kernel
    issued 2048 tiny 256 B descriptors that competed with the store
    stream);
  - each output tile u (rows {16p+u}) still stores as 128 contiguous
    16 KB descriptors.
The [T,T] transition is broadcast to all 128 partitions with four
0.5 MB stride-0 DRAM reads (two per HWDGE ring) dispatched before
anything else; the first output tile is split into two [128,2048]
sub-tiles whose adds depend only on the trb quarters they read (Tile
region deps), so the store stream launches ~8 us earlier than one
monolithic broadcast would allow. Steady-state tiles store whole
(2 MB, 16 KB descriptors) alternating between the two HWDGE rings —
measured ~26.9 GB/s per SDMA engine vs ~25.7 for 8 KB half-tile
descriptors and ~340 GB/s aggregate for v1's single-ring stores.
Alternatives tried and rejected: PE ones-matmul broadcast into PSUM
(fp32 matmul is 4-pass, ~1 us per 512-col bank, and the trailing
Tensor DRAIN gates the last reader); SWDGE partition_broadcast (Q7
serializes quarters behind a ~14 us drain and its shared-SBUF-port
use doubles concurrent DVE op durations). The kernel is store-bound;
the DVE add stream (~71 us) hides under the ~80-90 us store stream.
Residual variance across runs comes from the device, not the kernel:
some cores have one SDMA engine running ~20% slow (engine-local 0 or
15), and HBM-stack neighbor pairs that stream in lockstep throttle to
~358 GB/s each.
"""

import os
from contextlib import ExitStack

import numpy as np

B, L, T = 32, 512, 64
N_CORES = 8
B_PER = B // N_CORES          # 4 batches per core
R = B_PER * L                 # 2048 rows per core
P = 128                       # SBUF partitions
U = R // P                    # 16 rows per partition == tiles
TT = T * T                    # 4096
RAMP = 4                      # trb broadcast quarters
SUBW = TT // RAMP             # 1024
NSUB = 2                      # ramp sub-tiles for the first tile
SUBT = TT // NSUB             # 2048

_cache = {}

# Set by each kernel() call when tracing is enabled (BASS_KERNEL_TRACE=1):
# the BassKernelResults from run_bass_kernel_spmd, for harnesses that want
# exec_time_ns / trace paths.
last_results = None


def _patch_sem_clear():
    """Replace the raw-ISA EVENT_SEMAPHORE_RANGE_CLEAR (opcode 176) with
    per-sem EventSemaphore writes.

    The walrus build in this container rejects the RANGE_CLEAR encoding
    ("ISA wrong length" in visitInstISA); plain InstEventSemaphore with a
    sem-wr-imm update is lowered by walrus itself and is equivalent for
    the small ranges Tile resets.
    """
    import concourse.bass as bass
    import concourse.mybir as mybir

    if getattr(bass.BassGpSimd, "_sem_clear_patched", False):
        return

    def sem_clear(self, sem):
        nums = list(sem) if isinstance(sem, range) else [sem.num]
        last = None
        for n in nums:
            upd = mybir.SyncUpdate(
                sync_type="semaphore",
                id=n,
                update_mode="sem-wr-imm",
                update_value=0,
                ant_name=f"sem_{n}",
            )
            ins = mybir.InstEventSemaphore(
                name=self.bass.get_next_instruction_name(),
                ins=[],
                outs=[],
                sync_info=mybir.SyncInfo(on_wait=[], on_update=[upd]),
            )
            last = self.add_instruction(ins)
        return last

    for cls in (
        bass.BassGpSimd,
        bass.BassVectorEngine,
        bass.BassScalarEngine,
        bass.BassTensorEngine,
    ):
        cls.sem_clear = sem_clear
    bass.BassGpSimd._sem_clear_patched = True


def _build_bass():
    import concourse.bass as bass
    import concourse.mybir as mybir
    import concourse.tile as tile
    from concourse import bacc

    _patch_sem_clear()

    f32 = mybir.dt.float32
    nc = bacc.Bacc("TRN2", target_bir_lowering=False, debug=False)

    em = nc.dram_tensor("emission", [R, T], f32, kind="ExternalInput")
    tr = nc.dram_tensor("transition", [T, T], f32, kind="ExternalInput")
    out = nc.dram_tensor("out", [R, TT], f32, kind="ExternalOutput")

    # DRAM views for the p ↔ rows [16p, 16p+16) mapping.
    em_v = em[:].rearrange("(p u) j -> p (u j)", p=P)      # [128, 1024]
    out_v = out[:].rearrange("(p u) c -> p (u c)", p=P)    # [128, 65536]

    with ExitStack() as ctx:
        tc = ctx.enter_context(tile.TileContext(nc))
        const_pool = ctx.enter_context(tc.tile_pool(name="const", bufs=1))
        out_pool = ctx.enter_context(tc.tile_pool(name="out", bufs=4))
        ramp_pool = ctx.enter_context(tc.tile_pool(name="ramp", bufs=4))

        # Broadcast the flattened transition to all 128 partitions with
        # stride-0 DRAM-side APs, in 4 quarters split across both HWDGE
        # rings so the first quarter (which gates the first add) lands
        # early. (SWDGE partition_broadcast was tried instead and is
        # worse: the Q7 serializes the quarters at ~2 us each behind a
        # ~14 us drain, and its shared-SBUF-port use doubles concurrent
        # DVE tensor_tensor durations.) Emission loads first on the
        # scalar ring.
        em_all = const_pool.tile([P, U * T], f32)
        nc.scalar.dma_start(em_all[:], em_v)
        trb = const_pool.tile([P, TT], f32)
        tr_flat = tr[:].rearrange("a b -> (a b)").unsqueeze(0)
        for q in range(RAMP):
            ring = nc.sync if q % 2 == 0 else nc.scalar
            ring.dma_start(
                trb[:, bass.ts(q, SUBW)],
                tr_flat[:, bass.ts(q, SUBW)].broadcast_to([P, SUBW]),
            )

        def add(u, c0, w, tile_buf):
            ni = w // T
            nc.vector.tensor_add(
                tile_buf[:, :w].rearrange("p (i j) -> p i j", j=T),
                trb[:, c0 : c0 + w].rearrange("p (i j) -> p i j", j=T),
                em_all[:, bass.ts(u, T)].unsqueeze(1).broadcast_to([P, ni, T]),
            )

        # Ramp: tiles 0 and 1 go out as four [128,2048] sub-tiles (8 KB
        # descriptors, alternating rings) so the store stream starts as
        # soon as the first two trb quarters + emission have landed.
        rsel = 0
        for u in range(2):
            for q in range(NSUB):
                sub = ramp_pool.tile([P, SUBT], f32)
                add(u, q * SUBT, SUBT, sub)
                ring = nc.sync if rsel % 2 == 0 else nc.scalar
                rsel += 1
                base = u * TT + q * SUBT
                ring.dma_start(out_v[:, base : base + SUBT], sub[:])

        # Steady state: adjacent tiles (u, u+1) are consecutive rows of
        # every partition, so a merged store is 32 KB contiguous per
        # partition — half the packets of 16 KB whole-tile stores. Pairs
        # alternate rings; the final pair splits into its two 2 MB tiles
        # across both rings so the two completion receipts (which gate
        # the teardown barrier) overlap.
        for k, u in enumerate(range(2, U, 2)):
            big = out_pool.tile([P, 2 * TT], f32)
            add(u, 0, TT, big)
            nc.vector.tensor_add(
                big[:, TT : 2 * TT].rearrange("p (i j) -> p i j", j=T),
                trb[:].rearrange("p (i j) -> p i j", j=T),
                em_all[:, bass.ts(u + 1, T)].unsqueeze(1).broadcast_to([P, T, T]),
            )
            base = u * TT
            if u + 1 == U - 1:
                nc.sync.dma_start(out_v[:, base : base + TT], big[:, :TT])
                nc.scalar.dma_start(
                    out_v[:, base + TT : base + 2 * TT], big[:, TT:]
                )
            else:
                ring = nc.sync if k % 2 == 0 else nc.scalar
                ring.dma_start(out_v[:, base : base + 2 * TT], big[:])

    nc.compile()
    return nc


def _get_nc():
    if "nc" not in _cache:
        _cache["nc"] = _build_bass()
    return _cache["nc"]


def _ensure_ntff_hook():
    """bass_utils' trace path imports antenv.axon_hooks, which this image
    lacks. Register a stand-in built from trn_boot's ctypes NTFF hook so
    tracing works; degrade silently (bass_utils handles a None hook) if
    any piece is missing."""
    import sys
    import types

    try:
        import antenv.axon_hooks  # noqa: F401
        return
    except ImportError:
        pass
    try:
        import antenv  # noqa: F401
        from trn_agent_boot import trn_boot

        hook = trn_boot._ntff_profile_via_ctypes("/opt/axon/libaxon_pjrt.so")
    except Exception:
        hook = None
    mod = types.ModuleType("antenv.axon_hooks")
    mod.get_axon_ntff_profile_hook = lambda: hook
    mod.set_axon_ntff_profile_hook = lambda h: None
    sys.modules["antenv.axon_hooks"] = mod


def kernel(emission: np.ndarray, transition: np.ndarray) -> np.ndarray:
    global last_results
    from concourse.bass_utils import run_bass_kernel_spmd

    nc = _get_nc()

    em = np.ascontiguousarray(emission, dtype=np.float32).reshape(N_CORES, R, T)
    tr = np.ascontiguousarray(transition, dtype=np.float32)
    in_maps = [{"emission": em[i], "transition": tr} for i in range(N_CORES)]

    trace = bool(os.environ.get("BASS_KERNEL_TRACE"))
    if trace or os.environ.get("BASS_TRACE"):
        _ensure_ntff_hook()
    res = run_bass_kernel_spmd(
        nc, in_maps, core_ids=list(range(N_CORES)), trace=trace
    )
    if trace:
        last_results = res

    # The kernel writes every DRAM row at its natural offset (the
    # p ↔ rows [16p, 16p+16) interleave only shapes the SBUF-side access
    # patterns), so no host-side reorder is needed.
    full = np.stack([res.results[i]["out"] for i in range(N_CORES)])
    return full.reshape(B, L, T, T)
